# revision 21
# baseline (speedup 1.0000x reference)
"""Trainium2 Bass kernel for nn_ObjectContextBlock (v3: grouped softmax).

Math (per batch element b, data-parallel over B=8 across 8 cores):
  q = relu(W2q relu(W1q x)), x: (C=512, HW=16384)
  k = relu(W2k relu(W1k proxy)), v = relu(Wv proxy), proxy: (C, Kp=19)
  att = softmax(q^T k / sqrt(Kc)) over k;  out = relu(Wo (v att^T) + bo)

v3 changes vs v2 (154.6us):
  * softmax grouping G=4: sim for 4 chunks lands in ONE psum bank at
    32-partition offsets; ONE exp (128,512) per group (was 4x (19,512))
    and ONE block-diag-ones den matmul per group (was 4x 512-cyc mms).
  * den rides in the sim bank's spare rows 124:127 (written by PE after
    the exp read; copied out by ACT; psS double-buffered so the copy
    never collides with the next group's sim writes).
  * merged 2-bank epilogues: q1 (ACT), q2 (DVE), out m01 (ACT), out m23
    (DVE) each drain (128,2,512) in one instruction - halves the
    fixed-overhead count vs per-bank ops.
  * out halves alternate iterations (m01 lag 6, m23 lag 7) sharing one
    2-bank psum tag; PSUM = q1(2) + q2(2) + psS(2) + out(2) = 8 banks.
  * out + den DMA on gpsimd SWDGE; x prefetch one group ahead on SP.

Toolchain constraint (walrus build): at most ONE sync wait per instruction;
patched via single-wait drains + NoOp wait-splitting (same as baseline).
"""

import numpy as np
import ml_dtypes

import bass_rust as _br
import concourse.bass as bass
import concourse.mybir as mybir
import concourse.tile as tile
from concourse.bass import ds
from concourse.bass_utils import run_bass_kernel_spmd
from concourse.tile import TileContext

F32 = mybir.dt.float32
F32R = mybir.dt.float32r
FP8 = mybir.dt.float8e4
BF16 = mybir.dt.bfloat16
AF = mybir.ActivationFunctionType
ALU = mybir.AluOpType
DR = mybir.MatmulPerfMode.DoubleRow

E4NP = ml_dtypes.float8_e4m3

P = 128
C = 512          # input/output channels
KC = 256         # key channels
KP = 19          # proxy positions
KPP = 20         # proxy padded to even (f32r matmul moving dim must be even)
HW = 128 * 128   # spatial positions per batch
NT = 512         # chunk width
NCH = HW // NT   # 32 chunks
G = 4            # chunks per softmax group
NGR = NCH // G   # 8 groups
XG = 4           # x DMA group, chunks
OG = 4           # out DMA group, chunks
DG = 2           # den DMA group, groups
EPS = 1e-5
INV_STD = 1.0 / np.sqrt(1.0 + EPS)

S1 = 8.0         # q1 storage scale
S2 = 256.0       # q2 storage scale
EXPSC = 1.0 / (S2 * 16.0)   # exp scale: undoes S2 and Kc^-0.5=1/16


def _patched_drain_and_barrier(self, tick_clock, wait_clock):
    # walrus encodes at most ONE sync wait per instruction; emit one
    # single-wait drain per live proc instead of the stock multi-wait drain.
    gc = tick_clock.global_clock
    for p in range(_br.N_PROCS):
        v = gc[p]
        if v > 0:
            d = self.nc.sync.drain()
            vc = _br.VectorClock([v if q == p else 0 for q in range(_br.N_PROCS)])
            wait_clock.add_sem_waits(d.ins, _br.ScopedClock({None: vc}))
    self.nc.all_engine_barrier()
    popped = self.nc._tile_sem_poison_stack.pop()
    assert popped is self._sem_poison
    self.nc.clear_and_free_semaphores(list(self.sems.allocated().values()))
    self.nc.all_engine_barrier()


TileContext._drain_and_barrier = _patched_drain_and_barrier


def _split_multiwaits(bir_json: bytes) -> bytes:
    """Hoist extra sync waits onto NoOps just before the offender (same
    engine, in-order execution, so waiting earlier is equivalent)."""
    import orjson
    js = orjson.loads(bir_json)
    for fn in js["functions"]:
        for b in fn["blocks"]:
            out = []
            for ins in b["instructions"]:
                si = ins.get("sync_info")
                waits = (si or {}).get("on_wait") or []
                if len(waits) > 1:
                    for j, w in enumerate(waits[:-1]):
                        out.append({
                            "debug": ins.get("debug", 0),
                            "engine": ins["engine"],
                            "ins": [], "outs": [],
                            "name": f"{ins['name']}-wsplit{j}",
                            "opcode": "NoOp",
                            "sync_info": {"on_wait": [w], "on_update": []},
                        })
                    si["on_wait"] = [waits[-1]]
                out.append(ins)
            b["instructions"] = out
    return orjson.dumps(js)


import concourse.bass_utils as _bu
import concourse.bass2jax as _b2j

if not getattr(_bu, "_wsplit_patched", False):
    _orig_compile_bir = _bu.compile_bir_kernel

    def _compile_bir_split(bir_json, tmpdir, neff_name="file.neff"):
        return _orig_compile_bir(_split_multiwaits(bir_json), tmpdir, neff_name)

    _bu.compile_bir_kernel = _compile_bir_split
    _b2j.compile_bir_kernel = _compile_bir_split
    _bu._wsplit_patched = True


def build(zero_qbias=True, ncols=HW):
    """Single-core Bass module (SPMD across the 8 cores)."""
    nch = ncols // NT
    ngr = nch // G
    nc = bass.Bass("TRN2", debug=False)

    x = nc.dram_tensor("x", (C, ncols), FP8, kind="ExternalInput").ap()
    proxy = nc.dram_tensor("proxy", (C, KPP), F32R, kind="ExternalInput").ap()
    w1q = nc.dram_tensor("w1q", (C, KC), FP8, kind="ExternalInput").ap()    # fp8(S1*w1q^T)
    w2q = nc.dram_tensor("w2q", (KC, KC), FP8, kind="ExternalInput").ap()   # fp8((S2/S1)*w2q^T)
    w1k = nc.dram_tensor("w1k", (C, KC), F32R, kind="ExternalInput").ap()
    w2k = nc.dram_tensor("w2k", (KC, KC), F32R, kind="ExternalInput").ap()
    wv = nc.dram_tensor("wv", (C, KC), F32R, kind="ExternalInput").ap()
    wo = nc.dram_tensor("wo", (KC, C), F32R, kind="ExternalInput").ap()
    b1q = nc.dram_tensor("b1q", (P, KC // P), F32, kind="ExternalInput").ap()  # S1*bq1
    b2q = nc.dram_tensor("b2q", (P, KC // P), F32, kind="ExternalInput").ap()  # S2*bq2
    b1k = nc.dram_tensor("b1k", (P, KC // P), F32, kind="ExternalInput").ap()
    b2k = nc.dram_tensor("b2k", (P, KC // P), F32, kind="ExternalInput").ap()
    bvp = nc.dram_tensor("bvp", (P, KC // P), F32, kind="ExternalInput").ap()
    bor = nc.dram_tensor("bor", (1, C), F32R, kind="ExternalInput").ap()       # bo row
    bones = nc.dram_tensor("bones", (P, 32), BF16, kind="ExternalInput").ap()  # block-diag ones
    out = nc.dram_tensor("out", (C, ncols), BF16, kind="ExternalOutput").ap()
    den = nc.dram_tensor("den", (G, ngr * NT), F32, kind="ExternalOutput").ap()

    x_t = x.rearrange("(c p) n -> p c n", p=P)      # (128, 4, ncols)
    out_t = out.rearrange("(c p) n -> p c n", p=P)  # (128, 4, ncols)

    CK = C // P    # 4
    KK = KC // P   # 2
    CO = C // P    # 4

    from contextlib import ExitStack
    with TileContext(nc) as tc, ExitStack() as ctx:
        wpool = ctx.enter_context(tc.tile_pool(name="weights", bufs=1))
        xpool = ctx.enter_context(tc.tile_pool(name="xp", bufs=2))
        work = ctx.enter_context(tc.tile_pool(name="work", bufs=2))
        opool = ctx.enter_context(tc.tile_pool(name="op", bufs=2))
        psum = ctx.enter_context(tc.tile_pool(name="ps", bufs=1, space="PSUM"))

        def load(name, ap_in, shape, dt):
            t = wpool.tile(list(shape), dt, tag=f"w_{name}")
            nc.sync.dma_start(out=t, in_=ap_in)
            return t

        # first x group DMA before the weight loads (SP queue is FIFO)
        xg0 = xpool.tile([P, CK, XG * NT], FP8, tag="xg", bufs=3)
        nc.sync.dma_start(out=xg0, in_=x_t[:, :, ds(0, XG * NT)])

        w1q_sb = load("w1q", w1q.rearrange("(c p) m -> p c m", p=P), (P, CK, KC), FP8)
        w2q_sb = load("w2q", w2q.rearrange("(c p) m -> p c m", p=P), (P, KK, KC), FP8)
        w1k_sb = load("w1k", w1k.rearrange("(c p) m -> p c m", p=P), (P, CK, KC), F32R)
        w2k_sb = load("w2k", w2k.rearrange("(c p) m -> p c m", p=P), (P, KK, KC), F32R)
        wv_sb = load("wv", wv.rearrange("(c p) m -> p c m", p=P), (P, CK, KC), F32R)
        wo_sb = load("wo", wo.rearrange("(c p) m -> p c m", p=P), (P, KK, C), F32R)
        proxy_sb = load("proxy", proxy.rearrange("(c p) k -> p c k", p=P), (P, CK, KPP), F32R)
        bor_sb = load("bor", bor, (1, C), F32R)
        bones_sb = load("bones", bones, (P, 32), BF16)
        if not zero_qbias:
            b1q_sb = load("b1q", b1q, (P, KC // P), F32)
            b2q_sb = load("b2q", b2q, (P, KC // P), F32)
        b1k_sb = load("b1k", b1k, (P, KC // P), F32)
        b2k_sb = load("b2k", b2k, (P, KC // P), F32)
        bv_sb = load("bvp", bvp, (P, KC // P), F32)

        # constants (via ACT so consumers wait on one engine)
        ones1_20 = wpool.tile([1, KPP], F32R, tag="ones1_20")
        nc.scalar.copy(out=ones1_20, in_=nc.const_aps.tensor(1.0, (1, KPP)))

        # ---------- preamble: k-path, v, wov (tiny; reuses the psO tag) ----
        def pre_ps():
            t = psum.tile([P, 2, NT], F32, tag="psO", name="pre", bufs=1)
            return t[:, 0, :]

        # k1 = relu(w1k' proxy + b1k): (KC, KPP) f32r
        k1_sb = wpool.tile([P, KK, KPP], F32R, tag="k1s")
        for m in range(KK):
            pk = pre_ps()[:, :KPP]
            for c in range(CK):
                nc.tensor.matmul(pk, lhsT=w1k_sb[:, c, ds(m * P, P)],
                                 rhs=proxy_sb[:, c, :],
                                 start=(c == 0), stop=(c == CK - 1))
            nc.scalar.activation(out=k1_sb[:, m, :], in_=pk, func=AF.Relu,
                                 bias=b1k_sb[:, m:m + 1], scale=1.0)
        # k2 = relu(w2k' k1 + bk2): (KC, KPP) f32r, padded to 32 stationary
        # cols of zeros so each sim matmul writes its full 32-row psum block
        # (rows 19:32 become zeros -> no stale psum reads for exp).
        k2_sb = wpool.tile([P, KK, 32], BF16, tag="k2s")
        for m in range(KK):
            nc.scalar.copy(out=k2_sb[:, m, KPP:],
                           in_=nc.const_aps.tensor(0.0, (P, 32 - KPP)))
        for m in range(KK):
            pk = pre_ps()[:, :KPP]
            for c in range(KK):
                nc.tensor.matmul(pk, lhsT=w2k_sb[:, c, ds(m * P, P)],
                                 rhs=k1_sb[:, c, :],
                                 start=(c == 0), stop=(c == KK - 1))
            nc.scalar.activation(out=k2_sb[:, m, :KPP], in_=pk, func=AF.Relu,
                                 bias=b2k_sb[:, m:m + 1], scale=1.0)
        # v = relu(wv' proxy + bv): (KC, KPP) f32r
        v_sb = wpool.tile([P, KK, KPP], F32R, tag="vsb")
        for m in range(KK):
            pv = pre_ps()[:, :KPP]
            for c in range(CK):
                nc.tensor.matmul(pv, lhsT=wv_sb[:, c, ds(m * P, P)],
                                 rhs=proxy_sb[:, c, :],
                                 start=(c == 0), stop=(c == CK - 1))
            nc.scalar.activation(out=v_sb[:, m, :], in_=pv, func=AF.Relu,
                                 bias=bv_sb[:, m:m + 1], scale=1.0)
        # wovT[k, c_out] = sum_kc v[kc,k] wo[c_out,kc]  (+ bo on every row)
        pw = pre_ps()[:KPP, :]
        for c in range(KK):
            nc.tensor.matmul(pw, lhsT=v_sb[:, c, :], rhs=wo_sb[:, c, :],
                             start=(c == 0), stop=False, skip_group_check=True)
        nc.tensor.matmul(pw[:KPP, :], lhsT=ones1_20, rhs=bor_sb,
                         start=False, stop=True, skip_group_check=True)
        # replicate wovT to partition bases 0/32/64/96 so the out matmuls'
        # stationary base matches att6's row base (PE quadrant tiling).
        wovT_sb = wpool.tile([P, C], BF16, tag="wovT")
        for j in range(G):
            nc.scalar.copy(out=wovT_sb[ds(32 * j, KP), :], in_=pw[:KP, :])

        # ---------- main loop ----------
        xg = xg0
        og = None
        dstage = None
        psS_cur = None   # sim/den psum tile of the current group
        psS_byg = {}
        att6v = {}       # group -> att6 sbuf tile
        q1v = {}
        q2v = {}

        xg_next = None

        def q1_stage(i):
            # at each x-group boundary: promote the prefetched tile, then
            # issue the DMA for the group after it (one group of lead).
            nonlocal xg, xg_next
            if i % XG == 0:
                if i > 0:
                    xg = xg_next
                gnext = i // XG + 1
                if gnext * XG * NT < ncols:
                    xg_next = xpool.tile([P, CK, XG * NT], FP8, tag="xg", bufs=3)
                    nc.sync.dma_start(out=xg_next,
                                      in_=x_t[:, :, ds(gnext * XG * NT, XG * NT)])
            xr = xg[:, :, ds((i % XG) * NT, NT)]
            pq = psum.tile([P, KK, NT], F32, tag="psQ1", name="pq1", bufs=1)
            for m in range(KK):
                for j in range(2):
                    nc.tensor.matmul(pq[:, m, :],
                                     lhsT=w1q_sb[:, ds(2 * j, 2), ds(m * P, P)],
                                     rhs=xr[:, ds(2 * j, 2), :],
                                     start=(j == 0), stop=(j == 1), perf_mode=DR)
            q1s = work.tile([P, KK, NT], FP8, tag="q1s", bufs=3)
            if zero_qbias:
                nc.scalar.activation(out=q1s, in_=pq, func=AF.Relu, scale=1.0)
            else:
                nc.scalar.activation(out=q1s[:, 0, :], in_=pq[:, 0, :],
                                     func=AF.Relu, bias=b1q_sb[:, 0:1], scale=1.0)
                nc.scalar.activation(out=q1s[:, 1, :], in_=pq[:, 1, :],
                                     func=AF.Relu, bias=b1q_sb[:, 1:2], scale=1.0)
            return q1s

        def q2_stage(i, q1s):
            pq = psum.tile([P, KK, NT], F32, tag="psQ2", name="pq2", bufs=1)
            for m in range(KK):
                nc.tensor.matmul(pq[:, m, :],
                                 lhsT=w2q_sb[:, 0:2, ds(m * P, P)],
                                 rhs=q1s[:, 0:2, :],
                                 start=True, stop=True, perf_mode=DR)
            q2s = work.tile([P, KK, NT], BF16, tag="q2s", bufs=3)
            if zero_qbias:
                nc.vector.tensor_scalar_max(q2s, pq, 0.0)
            else:
                nc.vector.tensor_scalar(out=q2s[:, 0, :], in0=pq[:, 0, :],
                                        scalar1=b2q_sb[:, 0:1], scalar2=0.0,
                                        op0=ALU.add, op1=ALU.max)
                nc.vector.tensor_scalar(out=q2s[:, 1, :], in0=pq[:, 1, :],
                                        scalar1=b2q_sb[:, 1:2], scalar2=0.0,
                                        op0=ALU.add, op1=ALU.max)
            return q2s

        def sim_stage(i, q2s):
            # sim chunk i -> rows 32j..32j+18 of the group's psS bank
            nonlocal psS_cur
            j = i % G
            if j == 0:
                psS_cur = psum.tile([P, NT], F32, tag="psS", name="psS", bufs=2)
            pS = psS_cur[ds(32 * j, 32), :]
            for c in range(KK):
                nc.tensor.matmul(pS, lhsT=k2_sb[:, c, :],
                                 rhs=q2s[:, c, :],
                                 start=(c == 0), stop=(c == KK - 1),
                                 tile_position=(0, 32 * j))
            if j == G - 1:
                g = i // G
                psS_byg[g] = psS_cur
                att6 = work.tile([P, NT], BF16, tag="att6", bufs=2)
                nc.scalar.activation(out=att6, in_=psS_cur, func=AF.Exp,
                                     scale=EXPSC)
                att6v[g] = att6

        def den_stage(g):
            # den rows for group g -> psS rows 124:127 (efter exp's read);
            # copy into the staging tile; DMA every DG groups via gpsimd.
            nonlocal dstage
            pS = psS_byg.pop(g)
            # base-96 32-row output: rows 96:100 = den, rows 100:128 zeros
            # (bones cols 4:32 are zero); overwrites sim j=3's rows, which
            # exp already consumed.
            nc.tensor.matmul(pS[ds(0, 32), :], lhsT=bones_sb,
                             rhs=att6v[g], start=True, stop=True,
                             tile_position=(0, 0))
            if g % DG == 0:
                dstage = work.tile([G, DG * NT], F32, tag="densb", bufs=2)
            nc.scalar.copy(out=dstage[:, ds((g % DG) * NT, NT)],
                           in_=pS[ds(0, G), :])
            if g % DG == DG - 1:
                nc.gpsimd.dma_start(
                    out=den[:, ds((g - DG + 1) * NT, DG * NT)], in_=dstage)

        def out_half01(i):
            nonlocal og
            if i % OG == 0:
                og = opool.tile([P, CO, OG * NT], BF16, tag="osb", bufs=2)
            col = ds((i % OG) * NT, NT)
            att6 = att6v[i // G]
            po = psum.tile([P, 2, NT], F32, tag="psO", name="po01", bufs=1)
            for m in range(2):
                nc.tensor.matmul(po[:, m, :],
                                 lhsT=wovT_sb[ds(32 * (i % G), KP), ds(m * P, P)],
                                 rhs=att6[ds(32 * (i % G), KP), :],
                                 start=True, stop=True,
                                 tile_position=(32 * (i % G), 0))
            nc.scalar.activation(out=og[:, 0:2, col], in_=po, func=AF.Relu)

        def out_half23(i):
            col = ds((i % OG) * NT, NT)
            att6 = att6v[i // G]
            po = psum.tile([P, 2, NT], F32, tag="psO", name="po23", bufs=1)
            for m in range(2):
                nc.tensor.matmul(po[:, m, :],
                                 lhsT=wovT_sb[ds(32 * (i % G), KP), ds((m + 2) * P, P)],
                                 rhs=att6[ds(32 * (i % G), KP), :],
                                 start=True, stop=True,
                                 tile_position=(32 * (i % G), 0))
            nc.vector.tensor_scalar_max(og[:, 2:4, col], po, 0.0)
            if i % G == G - 1 and i // G >= 2:
                att6v.pop(i // G - 2, None)
            if i % OG == OG - 1:
                nc.gpsimd.dma_start(out=out_t[:, :, ds((i - OG + 1) * NT, OG * NT)],
                                    in_=og)

        # skew: out23(it-7) | q1(it) | q2(it-1) | sim+exp(it-2) | out01(it-6)
        # | den(group of it-6).  wovT is only ever read from partitions 0:19,
        # but att6 rows live at 32j offsets -> slice wovT per j via a
        # replicated copy.
        for it in range(nch + 8):
            j7 = it - 7
            if 0 <= j7 < nch:
                out_half23(j7)
            if it < nch:
                q1v[it] = q1_stage(it)
            j1 = it - 1
            if 0 <= j1 < nch:
                q2v[j1] = q2_stage(j1, q1v.pop(j1))
            j2 = it - 2
            if 0 <= j2 < nch:
                sim_stage(j2, q2v.pop(j2))
            j6 = it - 6
            if 0 <= j6 < nch:
                out_half01(j6)
                if j6 % G == 0:
                    den_stage(j6 // G)
    return nc


def _prep_inputs(x, proxy_feats, wq1, gq1, bq1, wq2, gq2, bq2,
                 wk1, gk1, bk1, wk2, gk2, bk2, wv, gv, bv, wo, go, bo):
    """Host-side: fold BN into weights/biases, apply fp8 scaling, transpose
    for lhsT layout, rearrange biases to per-partition layout."""
    def fold(w, g):
        return (w * (INV_STD * g)[:, None]).astype(np.float32)

    def part(b):  # (M,) -> (128, M//128) with [p, m] = b[m*128+p]
        return np.ascontiguousarray(np.asarray(b).reshape(-1, P).T.astype(np.float32))

    w1q_f = fold(wq1, gq1)   # (KC, C)
    w2q_f = fold(wq2, gq2)
    w1k_f = fold(wk1, gk1)
    w2k_f = fold(wk2, gk2)
    wv_f = fold(wv, gv)
    wo_f = fold(wo, go)      # (C, KC)

    bones = np.zeros((P, 32), np.float32)
    for j in range(G):
        bones[32 * j:32 * j + KP, j] = 1.0
    bones = bones.astype(ml_dtypes.bfloat16)

    common = {
        "w1q": np.ascontiguousarray((S1 * w1q_f).T).astype(E4NP),
        "w2q": np.ascontiguousarray(((S2 / S1) * w2q_f).T).astype(E4NP),
        "w1k": np.ascontiguousarray(w1k_f.T),
        "w2k": np.ascontiguousarray(w2k_f.T),
        "wv": np.ascontiguousarray(wv_f.T),
        "wo": np.ascontiguousarray(wo_f.T),
        "b1q": part(S1 * np.asarray(bq1)), "b2q": part(S2 * np.asarray(bq2)),
        "b1k": part(bk1), "b2k": part(bk2),
        "bvp": part(bv),
        "bor": np.ascontiguousarray(np.asarray(bo, np.float32).reshape(1, C)),
        "bones": bones,
    }
    B = x.shape[0]
    in_maps = []
    for b in range(B):
        m = dict(common)
        m["x"] = np.ascontiguousarray(x[b].reshape(C, -1)).astype(E4NP)
        pr = proxy_feats[b, :, :, 0].astype(np.float32)
        m["proxy"] = np.ascontiguousarray(np.pad(pr, ((0, 0), (0, KPP - KP))))
        in_maps.append(m)
    return in_maps


_NC_CACHE = {}


def kernel(**inputs):
    inputs = {k: np.asarray(v) for k, v in inputs.items()}
    B, _, H, W = inputs["x"].shape
    assert B == 8
    zero_qbias = (not np.any(inputs["bq1"])) and (not np.any(inputs["bq2"]))
    in_maps = _prep_inputs(**inputs)
    key = ("nc", zero_qbias)
    if key not in _NC_CACHE:
        _NC_CACHE[key] = build(zero_qbias=zero_qbias)
        _NC_CACHE["nc"] = _NC_CACHE[key]
    res = run_bass_kernel_spmd(_NC_CACHE[key], in_maps, core_ids=list(range(8)))
    outs = []
    for b in range(B):
        pre = np.asarray(res.results[b]["out"], dtype=np.float32)   # (C, HW)
        dn = np.asarray(res.results[b]["den"], dtype=np.float32)    # (G, NGR*NT)
        dn = dn.reshape(G, NGR, NT).transpose(1, 0, 2).reshape(1, HW)
        outs.append((pre / dn).reshape(C, H, W))
    return np.stack(outs)


# revision 22
# speedup vs baseline: 1.0311x; 1.0311x over previous
"""Trainium2 Bass kernel for nn_ObjectContextBlock (v3: grouped softmax).

Math (per batch element b, data-parallel over B=8 across 8 cores):
  q = relu(W2q relu(W1q x)), x: (C=512, HW=16384)
  k = relu(W2k relu(W1k proxy)), v = relu(Wv proxy), proxy: (C, Kp=19)
  att = softmax(q^T k / sqrt(Kc)) over k;  out = relu(Wo (v att^T) + bo)

v3 changes vs v2 (154.6us):
  * softmax grouping G=4: sim for 4 chunks lands in ONE psum bank at
    32-partition offsets; ONE exp (128,512) per group (was 4x (19,512))
    and ONE block-diag-ones den matmul per group (was 4x 512-cyc mms).
  * den rides in the sim bank's spare rows 124:127 (written by PE after
    the exp read; copied out by ACT; psS double-buffered so the copy
    never collides with the next group's sim writes).
  * merged 2-bank epilogues: q1 (ACT), q2 (DVE), out m01 (ACT), out m23
    (DVE) each drain (128,2,512) in one instruction - halves the
    fixed-overhead count vs per-bank ops.
  * out halves alternate iterations (m01 lag 6, m23 lag 7) sharing one
    2-bank psum tag; PSUM = q1(2) + q2(2) + psS(2) + out(2) = 8 banks.
  * out + den DMA on gpsimd SWDGE; x prefetch one group ahead on SP.

Toolchain constraint (walrus build): at most ONE sync wait per instruction;
patched via single-wait drains + NoOp wait-splitting (same as baseline).
"""

import numpy as np
import ml_dtypes

import bass_rust as _br
import concourse.bass as bass
import concourse.mybir as mybir
import concourse.tile as tile
from concourse.bass import ds
from concourse.bass_utils import run_bass_kernel_spmd
from concourse.tile import TileContext

F32 = mybir.dt.float32
F32R = mybir.dt.float32r
FP8 = mybir.dt.float8e4
BF16 = mybir.dt.bfloat16
AF = mybir.ActivationFunctionType
ALU = mybir.AluOpType
DR = mybir.MatmulPerfMode.DoubleRow

E4NP = ml_dtypes.float8_e4m3

P = 128
C = 512          # input/output channels
KC = 256         # key channels
KP = 19          # proxy positions
KPP = 20         # proxy padded to even (f32r matmul moving dim must be even)
HW = 128 * 128   # spatial positions per batch
NT = 512         # chunk width
NCH = HW // NT   # 32 chunks
G = 4            # chunks per softmax group
NGR = NCH // G   # 8 groups
XG = 4           # x DMA group, chunks
OG = 4           # out DMA group, chunks
DG = 2           # den DMA group, groups
EPS = 1e-5
INV_STD = 1.0 / np.sqrt(1.0 + EPS)

S1 = 8.0         # q1 storage scale
S2 = 256.0       # q2 storage scale
EXPSC = 1.0 / (S2 * 16.0)   # exp scale: undoes S2 and Kc^-0.5=1/16


def _patched_drain_and_barrier(self, tick_clock, wait_clock):
    # walrus encodes at most ONE sync wait per instruction; emit one
    # single-wait drain per live proc instead of the stock multi-wait drain.
    gc = tick_clock.global_clock
    for p in range(_br.N_PROCS):
        v = gc[p]
        if v > 0:
            d = self.nc.sync.drain()
            vc = _br.VectorClock([v if q == p else 0 for q in range(_br.N_PROCS)])
            wait_clock.add_sem_waits(d.ins, _br.ScopedClock({None: vc}))
    self.nc.all_engine_barrier()
    popped = self.nc._tile_sem_poison_stack.pop()
    assert popped is self._sem_poison
    self.nc.clear_and_free_semaphores(list(self.sems.allocated().values()))
    self.nc.all_engine_barrier()


TileContext._drain_and_barrier = _patched_drain_and_barrier


def _split_multiwaits(bir_json: bytes) -> bytes:
    """Hoist extra sync waits onto NoOps just before the offender (same
    engine, in-order execution, so waiting earlier is equivalent)."""
    import orjson
    js = orjson.loads(bir_json)
    for fn in js["functions"]:
        for b in fn["blocks"]:
            out = []
            for ins in b["instructions"]:
                si = ins.get("sync_info")
                waits = (si or {}).get("on_wait") or []
                if len(waits) > 1:
                    for j, w in enumerate(waits[:-1]):
                        out.append({
                            "debug": ins.get("debug", 0),
                            "engine": ins["engine"],
                            "ins": [], "outs": [],
                            "name": f"{ins['name']}-wsplit{j}",
                            "opcode": "NoOp",
                            "sync_info": {"on_wait": [w], "on_update": []},
                        })
                    si["on_wait"] = [waits[-1]]
                out.append(ins)
            b["instructions"] = out
    return orjson.dumps(js)


import concourse.bass_utils as _bu
import concourse.bass2jax as _b2j

if not getattr(_bu, "_wsplit_patched", False):
    _orig_compile_bir = _bu.compile_bir_kernel

    def _compile_bir_split(bir_json, tmpdir, neff_name="file.neff"):
        return _orig_compile_bir(_split_multiwaits(bir_json), tmpdir, neff_name)

    _bu.compile_bir_kernel = _compile_bir_split
    _b2j.compile_bir_kernel = _compile_bir_split
    _bu._wsplit_patched = True


def build(zero_qbias=True, ncols=HW):
    """Single-core Bass module (SPMD across the 8 cores)."""
    nch = ncols // NT
    ngr = nch // G
    nc = bass.Bass("TRN2", debug=False)

    x = nc.dram_tensor("x", (C, ncols), FP8, kind="ExternalInput").ap()
    proxy = nc.dram_tensor("proxy", (C, KPP), F32R, kind="ExternalInput").ap()
    w1q = nc.dram_tensor("w1q", (C, KC), FP8, kind="ExternalInput").ap()    # fp8(S1*w1q^T)
    w2q = nc.dram_tensor("w2q", (KC, KC), FP8, kind="ExternalInput").ap()   # fp8((S2/S1)*w2q^T)
    w1k = nc.dram_tensor("w1k", (C, KC), F32R, kind="ExternalInput").ap()
    w2k = nc.dram_tensor("w2k", (KC, KC), F32R, kind="ExternalInput").ap()
    wv = nc.dram_tensor("wv", (C, KC), F32R, kind="ExternalInput").ap()
    wo = nc.dram_tensor("wo", (KC, C), F32R, kind="ExternalInput").ap()
    b1q = nc.dram_tensor("b1q", (P, KC // P), F32, kind="ExternalInput").ap()  # S1*bq1
    b2q = nc.dram_tensor("b2q", (P, KC // P), F32, kind="ExternalInput").ap()  # S2*bq2
    b1k = nc.dram_tensor("b1k", (P, KC // P), F32, kind="ExternalInput").ap()
    b2k = nc.dram_tensor("b2k", (P, KC // P), F32, kind="ExternalInput").ap()
    bvp = nc.dram_tensor("bvp", (P, KC // P), F32, kind="ExternalInput").ap()
    bor = nc.dram_tensor("bor", (1, C), F32R, kind="ExternalInput").ap()       # bo row
    bones = nc.dram_tensor("bones", (P, 32), BF16, kind="ExternalInput").ap()  # block-diag ones
    out = nc.dram_tensor("out", (C, ncols), BF16, kind="ExternalOutput").ap()
    den = nc.dram_tensor("den", (G, ngr * NT), F32, kind="ExternalOutput").ap()

    x_t = x.rearrange("(c p) n -> p c n", p=P)      # (128, 4, ncols)
    out_t = out.rearrange("(c p) n -> p c n", p=P)  # (128, 4, ncols)

    CK = C // P    # 4
    KK = KC // P   # 2
    CO = C // P    # 4

    from contextlib import ExitStack
    with TileContext(nc) as tc, ExitStack() as ctx:
        wpool = ctx.enter_context(tc.tile_pool(name="weights", bufs=1))
        xpool = ctx.enter_context(tc.tile_pool(name="xp", bufs=2))
        work = ctx.enter_context(tc.tile_pool(name="work", bufs=2))
        opool = ctx.enter_context(tc.tile_pool(name="op", bufs=2))
        psum = ctx.enter_context(tc.tile_pool(name="ps", bufs=1, space="PSUM"))

        def load(name, ap_in, shape, dt):
            t = wpool.tile(list(shape), dt, tag=f"w_{name}")
            nc.sync.dma_start(out=t, in_=ap_in)
            return t

        # first x group DMA before the weight loads (SP queue is FIFO)
        xg0 = xpool.tile([P, CK, XG * NT], FP8, tag="xg", bufs=3)
        nc.sync.dma_start(out=xg0, in_=x_t[:, :, ds(0, XG * NT)])

        w1q_sb = load("w1q", w1q.rearrange("(c p) m -> p c m", p=P), (P, CK, KC), FP8)
        w2q_sb = load("w2q", w2q.rearrange("(c p) m -> p c m", p=P), (P, KK, KC), FP8)
        w1k_sb = load("w1k", w1k.rearrange("(c p) m -> p c m", p=P), (P, CK, KC), F32R)
        w2k_sb = load("w2k", w2k.rearrange("(c p) m -> p c m", p=P), (P, KK, KC), F32R)
        wv_sb = load("wv", wv.rearrange("(c p) m -> p c m", p=P), (P, CK, KC), F32R)
        wo_sb = load("wo", wo.rearrange("(c p) m -> p c m", p=P), (P, KK, C), F32R)
        proxy_sb = load("proxy", proxy.rearrange("(c p) k -> p c k", p=P), (P, CK, KPP), F32R)
        bor_sb = load("bor", bor, (1, C), F32R)
        bones_sb = load("bones", bones, (P, 32), BF16)
        if not zero_qbias:
            b1q_sb = load("b1q", b1q, (P, KC // P), F32)
            b2q_sb = load("b2q", b2q, (P, KC // P), F32)
        b1k_sb = load("b1k", b1k, (P, KC // P), F32)
        b2k_sb = load("b2k", b2k, (P, KC // P), F32)
        bv_sb = load("bvp", bvp, (P, KC // P), F32)

        # constants (via ACT so consumers wait on one engine)
        ones1_20 = wpool.tile([1, KPP], F32R, tag="ones1_20")
        nc.scalar.copy(out=ones1_20, in_=nc.const_aps.tensor(1.0, (1, KPP)))

        # ---------- preamble: k-path, v, wov (tiny; reuses the psO tag) ----
        def pre_ps():
            t = psum.tile([P, 2, NT], F32, tag="psO", name="pre", bufs=1)
            return t[:, 0, :]

        # k1 = relu(w1k' proxy + b1k): (KC, KPP) f32r
        k1_sb = wpool.tile([P, KK, KPP], F32R, tag="k1s")
        for m in range(KK):
            pk = pre_ps()[:, :KPP]
            for c in range(CK):
                nc.tensor.matmul(pk, lhsT=w1k_sb[:, c, ds(m * P, P)],
                                 rhs=proxy_sb[:, c, :],
                                 start=(c == 0), stop=(c == CK - 1))
            nc.scalar.activation(out=k1_sb[:, m, :], in_=pk, func=AF.Relu,
                                 bias=b1k_sb[:, m:m + 1], scale=1.0)
        # k2 = relu(w2k' k1 + bk2): (KC, KPP) f32r, padded to 32 stationary
        # cols of zeros so each sim matmul writes its full 32-row psum block
        # (rows 19:32 become zeros -> no stale psum reads for exp).
        k2_sb = wpool.tile([P, KK, 32], BF16, tag="k2s")
        for m in range(KK):
            nc.scalar.copy(out=k2_sb[:, m, KPP:],
                           in_=nc.const_aps.tensor(0.0, (P, 32 - KPP)))
        for m in range(KK):
            pk = pre_ps()[:, :KPP]
            for c in range(KK):
                nc.tensor.matmul(pk, lhsT=w2k_sb[:, c, ds(m * P, P)],
                                 rhs=k1_sb[:, c, :],
                                 start=(c == 0), stop=(c == KK - 1))
            nc.scalar.activation(out=k2_sb[:, m, :KPP], in_=pk, func=AF.Relu,
                                 bias=b2k_sb[:, m:m + 1], scale=1.0)
        # v = relu(wv' proxy + bv): (KC, KPP) f32r
        v_sb = wpool.tile([P, KK, KPP], F32R, tag="vsb")
        for m in range(KK):
            pv = pre_ps()[:, :KPP]
            for c in range(CK):
                nc.tensor.matmul(pv, lhsT=wv_sb[:, c, ds(m * P, P)],
                                 rhs=proxy_sb[:, c, :],
                                 start=(c == 0), stop=(c == CK - 1))
            nc.scalar.activation(out=v_sb[:, m, :], in_=pv, func=AF.Relu,
                                 bias=bv_sb[:, m:m + 1], scale=1.0)
        # wovT[k, c_out] = sum_kc v[kc,k] wo[c_out,kc]  (+ bo on every row)
        pw = pre_ps()[:KPP, :]
        for c in range(KK):
            nc.tensor.matmul(pw, lhsT=v_sb[:, c, :], rhs=wo_sb[:, c, :],
                             start=(c == 0), stop=False, skip_group_check=True)
        nc.tensor.matmul(pw[:KPP, :], lhsT=ones1_20, rhs=bor_sb,
                         start=False, stop=True, skip_group_check=True)
        # replicate wovT to partition bases 0/32/64/96 so the out matmuls'
        # stationary base matches att6's row base (PE quadrant tiling).
        wovT_sb = wpool.tile([P, C], BF16, tag="wovT")
        for j in range(G):
            nc.scalar.copy(out=wovT_sb[ds(32 * j, KP), :], in_=pw[:KP, :])

        # ---------- main loop ----------
        xg = xg0
        og = None
        dstage = None
        psS_cur = None   # sim/den psum tile of the current group
        psS_byg = {}
        att6v = {}       # group -> att6 sbuf tile
        q1v = {}
        q2v = {}

        xg_next = None

        def q1_stage(i):
            # at each x-group boundary: promote the prefetched tile, then
            # issue the DMA for the group after it (one group of lead).
            nonlocal xg, xg_next
            if i % XG == 0:
                if i > 0:
                    xg = xg_next
                gnext = i // XG + 1
                if gnext * XG * NT < ncols:
                    xg_next = xpool.tile([P, CK, XG * NT], FP8, tag="xg", bufs=3)
                    nc.sync.dma_start(out=xg_next,
                                      in_=x_t[:, :, ds(gnext * XG * NT, XG * NT)])
            xr = xg[:, :, ds((i % XG) * NT, NT)]
            pq = psum.tile([P, KK, NT], F32, tag="psQ1", name="pq1", bufs=1)
            for m in range(KK):
                for j in range(2):
                    nc.tensor.matmul(pq[:, m, :],
                                     lhsT=w1q_sb[:, ds(2 * j, 2), ds(m * P, P)],
                                     rhs=xr[:, ds(2 * j, 2), :],
                                     start=(j == 0), stop=(j == 1), perf_mode=DR)
            q1s = work.tile([P, KK, NT], FP8, tag="q1s", bufs=3)
            if zero_qbias:
                nc.scalar.activation(out=q1s, in_=pq, func=AF.Relu, scale=1.0)
            else:
                nc.scalar.activation(out=q1s[:, 0, :], in_=pq[:, 0, :],
                                     func=AF.Relu, bias=b1q_sb[:, 0:1], scale=1.0)
                nc.scalar.activation(out=q1s[:, 1, :], in_=pq[:, 1, :],
                                     func=AF.Relu, bias=b1q_sb[:, 1:2], scale=1.0)
            return q1s

        def q2_stage(i, q1s):
            pq = psum.tile([P, KK, NT], F32, tag="psQ2", name="pq2", bufs=1)
            for m in range(KK):
                nc.tensor.matmul(pq[:, m, :],
                                 lhsT=w2q_sb[:, 0:2, ds(m * P, P)],
                                 rhs=q1s[:, 0:2, :],
                                 start=True, stop=True, perf_mode=DR)
            q2s = work.tile([P, KK, NT], BF16, tag="q2s", bufs=3)
            if zero_qbias:
                nc.vector.tensor_scalar_max(q2s, pq, 0.0)
            else:
                nc.vector.tensor_scalar(out=q2s[:, 0, :], in0=pq[:, 0, :],
                                        scalar1=b2q_sb[:, 0:1], scalar2=0.0,
                                        op0=ALU.add, op1=ALU.max)
                nc.vector.tensor_scalar(out=q2s[:, 1, :], in0=pq[:, 1, :],
                                        scalar1=b2q_sb[:, 1:2], scalar2=0.0,
                                        op0=ALU.add, op1=ALU.max)
            return q2s

        def sim_stage(i, q2s):
            # sim chunk i -> rows 32j..32j+18 of the group's psS bank
            nonlocal psS_cur
            j = i % G
            if j == 0:
                psS_cur = psum.tile([P, NT], F32, tag="psS", name="psS", bufs=2)
            pS = psS_cur[ds(32 * j, 32), :]
            for c in range(KK):
                nc.tensor.matmul(pS, lhsT=k2_sb[:, c, :],
                                 rhs=q2s[:, c, :],
                                 start=(c == 0), stop=(c == KK - 1),
                                 tile_position=(0, 32 * j))
            if j == G - 1:
                g = i // G
                psS_byg[g] = psS_cur
                att6 = work.tile([P, NT], BF16, tag="att6", bufs=2)
                nc.scalar.activation(out=att6, in_=psS_cur, func=AF.Exp,
                                     scale=EXPSC)
                att6v[g] = att6

        def den_stage(g):
            # den rows for group g -> psS rows 124:127 (efter exp's read);
            # copy into the staging tile; DMA every DG groups via gpsimd.
            nonlocal dstage
            pS = psS_byg.pop(g)
            # base-96 32-row output: rows 96:100 = den, rows 100:128 zeros
            # (bones cols 4:32 are zero); overwrites sim j=3's rows, which
            # exp already consumed.
            nc.tensor.matmul(pS[ds(0, 32), :], lhsT=bones_sb,
                             rhs=att6v[g], start=True, stop=True,
                             tile_position=(0, 0))
            if g % DG == 0:
                dstage = work.tile([G, DG * NT], F32, tag="densb", bufs=2)
            nc.scalar.copy(out=dstage[:, ds((g % DG) * NT, NT)],
                           in_=pS[ds(0, G), :])
            if g % DG == DG - 1:
                nc.gpsimd.dma_start(
                    out=den[:, ds((g - DG + 1) * NT, DG * NT)], in_=dstage)

        def out_half01(i):
            # m0/m1 GEMM halves; epilogue on DVE
            nonlocal og
            if i % OG == 0:
                og = opool.tile([P, CO, OG * NT], BF16, tag="osb", bufs=2)
            col = ds((i % OG) * NT, NT)
            att6 = att6v[i // G]
            po = psum.tile([P, 2, NT], F32, tag="psO", name="po01", bufs=1)
            for m in range(2):
                nc.tensor.matmul(po[:, m, :],
                                 lhsT=wovT_sb[ds(32 * (i % G), KP), ds(m * P, P)],
                                 rhs=att6[ds(32 * (i % G), KP), :],
                                 start=True, stop=True,
                                 tile_position=(32 * (i % G), 0))
            nc.vector.tensor_scalar_max(og[:, 0:2, col], po, 0.0)

        def out_half23(i):
            # m2/m3 GEMM halves; epilogue on ACT
            col = ds((i % OG) * NT, NT)
            att6 = att6v[i // G]
            po = psum.tile([P, 2, NT], F32, tag="psO", name="po23", bufs=1)
            for m in range(2):
                nc.tensor.matmul(po[:, m, :],
                                 lhsT=wovT_sb[ds(32 * (i % G), KP), ds((m + 2) * P, P)],
                                 rhs=att6[ds(32 * (i % G), KP), :],
                                 start=True, stop=True,
                                 tile_position=(32 * (i % G), 0))
            nc.scalar.activation(out=og[:, 2:4, col], in_=po, func=AF.Relu)
            if i % G == G - 1 and i // G >= 2:
                att6v.pop(i // G - 2, None)
            if i % OG == OG - 1:
                nc.gpsimd.dma_start(out=out_t[:, :, ds((i - OG + 1) * NT, OG * NT)],
                                    in_=og)

        # skew: q1(it) | q2(it-1) | sim(it-3), exp at group end | out01(it-7)
        # | out23(it-8) | den(group of it-7).
        # Emission order per iter is tuned so each engine's queue order
        # matches consumer order: on group-last iters sim+exp go FIRST (exp
        # early in ACT's queue); out23's ACT epilogue is emitted early (it
        # gates out01's psum pair); out01's DVE epilogue late (gates next
        # iter's out23).
        for it in range(nch + 9):
            j3 = it - 3
            grp_last = 0 <= j3 < nch and j3 % G == G - 1
            if grp_last:
                sim_stage(j3, q2v.pop(j3))
            j8 = it - 8
            if 0 <= j8 < nch:
                out_half23(j8)
            if it < nch:
                q1v[it] = q1_stage(it)
            j1 = it - 1
            if 0 <= j1 < nch:
                q2v[j1] = q2_stage(j1, q1v.pop(j1))
            if 0 <= j3 < nch and not grp_last:
                sim_stage(j3, q2v.pop(j3))
            j7 = it - 7
            if 0 <= j7 < nch:
                out_half01(j7)
                if j7 % G == 0:
                    den_stage(j7 // G)
    return nc


def _prep_inputs(x, proxy_feats, wq1, gq1, bq1, wq2, gq2, bq2,
                 wk1, gk1, bk1, wk2, gk2, bk2, wv, gv, bv, wo, go, bo):
    """Host-side: fold BN into weights/biases, apply fp8 scaling, transpose
    for lhsT layout, rearrange biases to per-partition layout."""
    def fold(w, g):
        return (w * (INV_STD * g)[:, None]).astype(np.float32)

    def part(b):  # (M,) -> (128, M//128) with [p, m] = b[m*128+p]
        return np.ascontiguousarray(np.asarray(b).reshape(-1, P).T.astype(np.float32))

    w1q_f = fold(wq1, gq1)   # (KC, C)
    w2q_f = fold(wq2, gq2)
    w1k_f = fold(wk1, gk1)
    w2k_f = fold(wk2, gk2)
    wv_f = fold(wv, gv)
    wo_f = fold(wo, go)      # (C, KC)

    bones = np.zeros((P, 32), np.float32)
    for j in range(G):
        bones[32 * j:32 * j + KP, j] = 1.0
    bones = bones.astype(ml_dtypes.bfloat16)

    common = {
        "w1q": np.ascontiguousarray((S1 * w1q_f).T).astype(E4NP),
        "w2q": np.ascontiguousarray(((S2 / S1) * w2q_f).T).astype(E4NP),
        "w1k": np.ascontiguousarray(w1k_f.T),
        "w2k": np.ascontiguousarray(w2k_f.T),
        "wv": np.ascontiguousarray(wv_f.T),
        "wo": np.ascontiguousarray(wo_f.T),
        "b1q": part(S1 * np.asarray(bq1)), "b2q": part(S2 * np.asarray(bq2)),
        "b1k": part(bk1), "b2k": part(bk2),
        "bvp": part(bv),
        "bor": np.ascontiguousarray(np.asarray(bo, np.float32).reshape(1, C)),
        "bones": bones,
    }
    B = x.shape[0]
    in_maps = []
    for b in range(B):
        m = dict(common)
        m["x"] = np.ascontiguousarray(x[b].reshape(C, -1)).astype(E4NP)
        pr = proxy_feats[b, :, :, 0].astype(np.float32)
        m["proxy"] = np.ascontiguousarray(np.pad(pr, ((0, 0), (0, KPP - KP))))
        in_maps.append(m)
    return in_maps


_NC_CACHE = {}


def kernel(**inputs):
    inputs = {k: np.asarray(v) for k, v in inputs.items()}
    B, _, H, W = inputs["x"].shape
    assert B == 8
    zero_qbias = (not np.any(inputs["bq1"])) and (not np.any(inputs["bq2"]))
    in_maps = _prep_inputs(**inputs)
    key = ("nc", zero_qbias)
    if key not in _NC_CACHE:
        _NC_CACHE[key] = build(zero_qbias=zero_qbias)
        _NC_CACHE["nc"] = _NC_CACHE[key]
    res = run_bass_kernel_spmd(_NC_CACHE[key], in_maps, core_ids=list(range(8)))
    outs = []
    for b in range(B):
        pre = np.asarray(res.results[b]["out"], dtype=np.float32)   # (C, HW)
        dn = np.asarray(res.results[b]["den"], dtype=np.float32)    # (G, NGR*NT)
        dn = dn.reshape(G, NGR, NT).transpose(1, 0, 2).reshape(1, HW)
        outs.append((pre / dn).reshape(C, H, W))
    return np.stack(outs)
